# revision 18
# baseline (speedup 1.0000x reference)
"""MoDA attention Trainium2 kernel, 8-way head-parallel (v2).

Sharding: core c owns Q heads {2c, 2c+1} and K/V head c (their GQA group),
plus that K head's depth caches. Each core computes its heads' projections,
RoPE, joint seq+depth softmax attention, and a partial output projection
(rows 2c*128:(2c+2)*128 of Wo). Host sums the 8 partial outputs.

v2 design notes (vs v1 baseline):
- Phase A: K / Q0 / Q1 kt-outer passes with 4 psum banks each, V packed
  4-blocks-per-bank, first matmul starts ~2us in (wk + xT row0 only).
- Depth path precomputed in phase A: eu (DVE), dl (PE ones-matmul),
  wd=exp(dl) into a [1, L*TC] row, row->128-partition broadcast via DMA
  (not GpSimd), t_acc = sum_l vdT_l*wd_l on GpSimd, wdsum rows for Z.
- Phase B software pipeline per unit k: S(k) -> outproj filler ->
  Z(k-1)+O(k-1) -> epilogue(k-2), sized so Scalar exps never stall PE.
  PSUM: psS 3 + psO 2 + psZ 1 + psOut 2 = 8 banks.
- Epilogue is 2 DVE ops (o_ps + t_acc, * zinv-broadcast).
- Out tiles batched [128, DM] -> 16 output DMAs.
"""

import os
import sys

sys.path.insert(0, "/opt/trn_rl_repo")

import numpy as np
import ml_dtypes

import concourse.bass as bass
import concourse.tile as tile
import concourse.mybir as mybir
from concourse import bacc
from concourse.bass_utils import run_bass_kernel_spmd

BF16 = mybir.dt.bfloat16
FP32 = mybir.dt.float32
NPBF16 = ml_dtypes.bfloat16

HQ, HK, HD, DM = 16, 8, 128, 2048
L = 4
GQA = HQ // HK
SCALE = float(HD) ** -0.5
N_CORES = 8
NQH = 2  # Q heads per core
TC = 512  # T chunk (free dim of most matmuls)
DK = DM // 128  # contraction tiles

_programs = {}
last_result = None


def _ts(i, n):
    return bass.ts(i, n)


def build_program(T):
    nc = bacc.Bacc(
        "TRN2",
        target_bir_lowering=False,
        debug=False,
        enable_asserts=False,
        num_devices=N_CORES,
    )

    xT = nc.dram_tensor("xT", [DM, T], BF16, kind="ExternalInput").ap()
    wq = nc.dram_tensor("wq", [128, DK, NQH * HD], BF16, kind="ExternalInput").ap()
    wk = nc.dram_tensor("wk", [128, DK, HD], BF16, kind="ExternalInput").ap()
    wv = nc.dram_tensor("wv", [128, DK, HD], BF16, kind="ExternalInput").ap()
    wo = nc.dram_tensor("wo", [128, NQH, DM], BF16, kind="ExternalInput").ap()
    cosT = nc.dram_tensor("cosT", [HD, T], BF16, kind="ExternalInput").ap()
    sinT = nc.dram_tensor("sinT", [HD, T], BF16, kind="ExternalInput").ap()
    kdT = nc.dram_tensor("kdT", [L, HD, T], BF16, kind="ExternalInput").ap()
    vdT = nc.dram_tensor("vdT", [L, HD, T], BF16, kind="ExternalInput").ap()
    mask = nc.dram_tensor("mask", [128, 128], BF16, kind="ExternalInput").ap()
    out = nc.dram_tensor("out", [T, DM], BF16, kind="ExternalOutput").ap()
    NU_ = (T // TC) * NQH
    wdd = nc.dram_tensor("wdd", [NU_, L, TC], BF16, kind="Internal").ap()

    NCH = T // TC  # 4 chunks
    NTB = T // 128  # 16 blocks
    NU = NCH * NQH  # 8 units

    with tile.TileContext(nc) as tc:
        with tc.tile_pool(name="persist", bufs=1) as cp:
            # ---- persistent SBUF ----
            wq_sb = cp.tile([128, DK, NQH * HD], BF16)
            wk_sb = cp.tile([128, DK, HD], BF16)
            wv_sb = cp.tile([128, DK, HD], BF16)
            wo_sb = cp.tile([128, NQH, DM], BF16)
            mask_sb = cp.tile([128, 128], BF16)
            ones_sb = cp.tile([128, 128], BF16)
            nc.vector.memset(ones_sb[:], 1.0)
            qT_sb = cp.tile([128, NQH, T], BF16)
            kT_sb = cp.tile([128, T], BF16)
            v_sb = cp.tile([128, NTB, HD], BF16)
            oT_sb = cp.tile([128, NQH, T], BF16)
            tacc_sb = cp.tile([128, NU, TC], BF16)  # depth contrib per unit
            wd4p_sb = [cp.tile([4, TC], BF16, name=f"wd4p{u}") for u in range(NU)]

            with tc.tile_pool(name="phA_sb", bufs=1) as ap_, \
                 tc.tile_pool(name="psA", bufs=8, space="PSUM") as psA, \
                 tc.tile_pool(name="sRope", bufs=2) as sR, \
                 tc.tile_pool(name="sEu", bufs=3) as sEu, \
                 tc.tile_pool(name="sBc", bufs=2) as sBc, \
                 tc.tile_pool(name="sTt", bufs=2) as sTt:
                xT_sb = ap_.tile([128, DK, T], BF16)
                cos_sb = ap_.tile([128, T], BF16)
                sin_sb = ap_.tile([128, T], BF16)
                kdT_sb = ap_.tile([128, L, T], BF16)
                vdT_sb = ap_.tile([128, L, T], BF16)

                # ---- input DMAs: sync queue feeds the proj pipeline in
                # consumption order; scalar queue brings rope/depth data ----
                nc.sync.dma_start(wk_sb[:, 0, :], wk[:, 0, :])
                nc.sync.dma_start(xT_sb[:, 0, 0:TC], xT[_ts(0, 128), 0:TC])
                nc.sync.dma_start(xT_sb[:, 0, TC:T], xT[_ts(0, 128), TC:T])
                nc.sync.dma_start(wk_sb[:, 1:DK, :], wk[:, 1:DK, :])
                nc.sync.dma_start(wq_sb[:], wq[:])
                for kt in range(1, DK):
                    eng = nc.sync if kt % 2 == 0 else nc.scalar
                    eng.dma_start(xT_sb[:, kt, :], xT[_ts(kt, 128), :])
                nc.sync.dma_start(wv_sb[:], wv[:])
                nc.sync.dma_start(wo_sb[:], wo[:])
                nc.sync.dma_start(mask_sb[:], mask[:])
                nc.scalar.dma_start(cos_sb[:], cosT[:])
                nc.scalar.dma_start(sin_sb[:], sinT[:])
                for l in range(L):
                    nc.scalar.dma_start(kdT_sb[:, l, :], kdT[l])
                for l in range(L):
                    nc.scalar.dma_start(vdT_sb[:, l, :], vdT[l])

                def rope_chunk(ps, dst, c):
                    # dst = ps*cos + rotate_half(ps)*sin, all [128, TC]
                    cs = cos_sb[:, _ts(c, TC)]
                    sn = sin_sb[:, _ts(c, TC)]
                    praw = sR.tile([128, TC], BF16, tag="praw")
                    nc.scalar.copy(praw[:], ps[:])  # frees the psum bank fast
                    prot = sR.tile([128, TC], BF16, tag="prot")
                    nc.vector.tensor_copy(prot[0:64, :], praw[64:128, :])
                    nc.vector.tensor_copy(prot[64:128, :], praw[0:64, :])
                    t1 = sR.tile([128, TC], BF16, tag="t1")
                    nc.vector.tensor_mul(t1[:], praw[:], cs)
                    t2 = sR.tile([128, TC], BF16, tag="t2")
                    nc.vector.tensor_mul(t2[:], prot[:], sn)
                    nc.vector.tensor_tensor(
                        dst[0:64, :], t1[0:64, :], t2[0:64, :],
                        op=mybir.AluOpType.subtract,
                    )
                    nc.vector.tensor_add(dst[64:128, :], t1[64:128, :], t2[64:128, :])

                # ---- pass 1: K proj (kt-outer) + V proj interleaved per
                # xT row; V uses a [128, 4, 128] layout in each bank so each
                # bank holds one whole-row group (no interleaved start/stop
                # column groups: V matmuls write [128, 512] spans via a
                # blocked rhs). ----
                kps = [psA.tile([128, TC], FP32, tag="a", name=f"kps{c}") for c in range(NCH)]
                q0ps = [psA.tile([128, TC], FP32, tag="a", name=f"q0ps{c}") for c in range(NCH)]
                for kt in range(DK):
                    for c in range(NCH):
                        nc.tensor.matmul(
                            kps[c][:], wk_sb[:, kt, :], xT_sb[:, kt, _ts(c, TC)],
                            start=(kt == 0), stop=(kt == DK - 1),
                        )
                    for c in range(NCH):
                        nc.tensor.matmul(
                            q0ps[c][:], wq_sb[:, kt, 0:HD], xT_sb[:, kt, _ts(c, TC)],
                            start=(kt == 0), stop=(kt == DK - 1),
                        )
                for c in range(NCH):
                    rope_chunk(kps[c], kT_sb[:, _ts(c, TC)], c)
                for c in range(NCH):
                    rope_chunk(q0ps[c], qT_sb[:, 0, _ts(c, TC)], c)

                # ---- pass 2: V proj, 4 t-blocks packed per psum bank ----
                for g in range(NCH):
                    vp = psA.tile([128, TC], FP32, tag="a", name=f"vp{g}")
                    for tbl in range(4):
                        tb = g * 4 + tbl
                        for kt in range(DK):
                            nc.tensor.matmul(
                                vp[:, _ts(tbl, 128)],
                                xT_sb[:, kt, _ts(tb, 128)], wv_sb[:, kt, :],
                                start=(kt == 0), stop=(kt == DK - 1),
                            )
                    if g % 2 == 0:
                        nc.scalar.copy(v_sb[:, g * 4:(g + 1) * 4, :], vp[:])
                    else:
                        nc.vector.tensor_copy(v_sb[:, g * 4:(g + 1) * 4, :], vp[:])

                # depth precompute for unit (c, h): eu (DVE) -> dl (PE)
                # -> wd=exp (Scalar) -> DRAM roundtrip bcast -> tacc (DVE)
                def depth_unit(c, h):
                    # dl matmul vs an all-ones [128,128] stationary operand:
                    # every psum row gets Z_l, so exp yields the broadcast
                    # wd_l tile directly (no DMA broadcast needed).
                    u = c * NQH + h
                    bcs = sBc.tile([128, L, TC], BF16, tag="bcs")
                    for l in range(L):
                        eu = sEu.tile([128, TC], BF16, tag="eu")
                        nc.vector.tensor_mul(
                            eu[:], qT_sb[:, h, _ts(c, TC)],
                            kdT_sb[:, l, _ts(c, TC)],
                        )
                        dlp = psA.tile([128, TC], FP32, tag="a", name=f"dl{u}_{l}")
                        nc.tensor.matmul(
                            dlp[:], ones_sb[:], eu[:], start=True, stop=True
                        )
                        nc.scalar.activation(
                            bcs[:, l, :], dlp[:],
                            mybir.ActivationFunctionType.Exp, scale=SCALE,
                        )
                    nc.sync.dma_start(wdd[u], bcs[0:1, :, :])
                    nc.sync.dma_start(wd4p_sb[u][:], wdd[u])
                    # tacc = sum_l vdT_l * wd_l  (batched DVE)
                    tmp4 = sTt.tile([128, L, TC], BF16, tag="tmp4")
                    nc.vector.tensor_mul(
                        tmp4[:], vdT_sb[:, :, _ts(c, TC)], bcs[:]
                    )
                    ta2 = sTt.tile([128, TC], BF16, tag="ta2")
                    nc.vector.tensor_add(ta2[:], tmp4[:, 0, :], tmp4[:, 1, :])
                    ta3 = sTt.tile([128, TC], BF16, tag="ta3")
                    nc.vector.tensor_add(ta3[:], tmp4[:, 2, :], tmp4[:, 3, :])
                    nc.vector.tensor_add(tacc_sb[:, u, :], ta2[:], ta3[:])

                # ---- pass 4: Q1 proj; per-chunk rope + depth follow ----
                q1ps = [psA.tile([128, TC], FP32, tag="a", name=f"q1ps{c}") for c in range(NCH)]
                for kt in range(DK):
                    for c in range(NCH):
                        nc.tensor.matmul(
                            q1ps[c][:], wq_sb[:, kt, HD:2 * HD], xT_sb[:, kt, _ts(c, TC)],
                            start=(kt == 0), stop=(kt == DK - 1),
                        )
                for c in range(NCH):
                    rope_chunk(q1ps[c], qT_sb[:, 1, _ts(c, TC)], c)
                for c in range(NCH):
                    depth_unit(c, 0)
                    depth_unit(c, 1)

            # ---- phase B: attention with software pipeline ----
            with tc.tile_pool(name="psS", bufs=3, space="PSUM") as psS, \
                 tc.tile_pool(name="psO", bufs=2, space="PSUM") as psO, \
                 tc.tile_pool(name="psZ", bufs=1, space="PSUM") as psZ, \
                 tc.tile_pool(name="psOut", bufs=2, space="PSUM") as psOut, \
                 tc.tile_pool(name="sU", bufs=34) as sU, \
                 tc.tile_pool(name="sZb", bufs=2) as sZb, \
                 tc.tile_pool(name="sOs", bufs=2) as sOs, \
                 tc.tile_pool(name="sRes", bufs=2) as sRes:

                units = [(c, h) for c in range(NCH) for h in range(NQH)]

                def s_phase(k):
                    c, h = units[k]
                    jmax = (c + 1) * 4
                    c0 = c * 4
                    us = []
                    for jb in range(jmax):
                        off = max(0, jb - c0) * 128
                        sp = psS.tile([128, TC], FP32, tag="s")
                        nc.tensor.matmul(
                            sp[:, off:TC], kT_sb[:, _ts(jb, 128)],
                            qT_sb[:, h, c * TC + off:(c + 1) * TC],
                            start=True, stop=True,
                        )
                        uu = sU.tile([128, TC], BF16, tag="u")
                        nc.scalar.activation(
                            uu[:, off:TC], sp[:, off:TC],
                            mybir.ActivationFunctionType.Exp, scale=SCALE,
                        )
                        if jb >= c0:
                            nc.vector.tensor_mul(
                                uu[:, off:off + 128], uu[:, off:off + 128],
                                mask_sb[:],
                            )
                        us.append((jb, off, uu))
                    return us

                def z_phase(k, us):
                    zp = psZ.tile([128, TC], FP32, tag="z")
                    for jb, off, uu in us:
                        nc.tensor.matmul(
                            zp[:, off:TC], ones_sb[:], uu[:, off:TC],
                            start=(jb == 0), stop=False,
                        )
                    nc.tensor.matmul(
                        zp[:], ones_sb[0:4, :], wd4p_sb[k][:],
                        start=False, stop=True,
                    )
                    zb = sZb.tile([128, TC], FP32, tag="zb")
                    nc.vector.reciprocal_approx_fast(zb[:], zp[:])
                    return zb

                def o_phase(k, us):
                    op = psO.tile([128, TC], FP32, tag="o")
                    for jb, off, uu in us:
                        nc.tensor.matmul(
                            op[:, off:TC], v_sb[:, jb, :], uu[:, off:TC],
                            start=(jb == 0), stop=(jb == len(us) - 1),
                        )
                    return op

                def epilogue(k, op, zb):
                    c, h = units[k]
                    osum = sOs.tile([128, TC], FP32, tag="osum")
                    nc.vector.tensor_add(osum[:], op[:], tacc_sb[:, k, :])
                    nc.vector.tensor_mul(
                        oT_sb[:, h, _ts(c, TC)], osum[:], zb[:]
                    )

                ncopy = [0]

                def outproj_tb(tb, split_dma=False):
                    res = sRes.tile([128, DM], BF16, tag="res")
                    for nch in range(DM // TC):
                        opp = psOut.tile([128, TC], FP32, tag="op")
                        for h in range(NQH):
                            nc.tensor.matmul(
                                opp[:], oT_sb[:, h, _ts(tb, 128)],
                                wo_sb[:, h, _ts(nch, TC)],
                                start=(h == 0), stop=(h == NQH - 1),
                            )
                        ncopy[0] += 1
                        if ncopy[0] % 2 == 0:
                            nc.scalar.copy(res[:, _ts(nch, TC)], opp[:])
                        else:
                            nc.vector.tensor_copy(res[:, _ts(nch, TC)], opp[:])
                        if split_dma:
                            nc.sync.dma_start(
                                out[_ts(tb, 128), _ts(nch, TC)], res[:, _ts(nch, TC)]
                            )
                    if not split_dma:
                        nc.sync.dma_start(out[_ts(tb, 128), :], res[:])

                pend_out = []

                def drain_out(n):
                    while n > 0 and pend_out:
                        outproj_tb(pend_out.pop(0))
                        n -= 1

                saved = {}
                for k in range(NU):
                    saved[k] = (s_phase(k),)
                    drain_out(1)
                    if k > 0:
                        us_prev = saved[k - 1][0]
                        zb = z_phase(k - 1, us_prev)
                        drain_out(1)
                        op = o_phase(k - 1, us_prev)
                        saved[k - 1] = (op, zb)
                    if k > 1:
                        op, zb = saved.pop(k - 2)
                        epilogue(k - 2, op, zb)
                        c2, h2 = units[k - 2]
                        if h2 == NQH - 1:
                            pend_out.extend(range(c2 * 4, (c2 + 1) * 4))
                # drain: unit 7 z/o, epilogues 6, 7, remaining outproj
                us7 = saved[7][0]
                zb7 = z_phase(7, us7)
                drain_out(2)
                op7 = o_phase(7, us7)
                op6, zb6 = saved.pop(6)
                epilogue(6, op6, zb6)
                drain_out(2)
                epilogue(7, op7, zb7)
                for tb in range(12, 16):
                    outproj_tb(tb, split_dma=(tb == 15))

    nc.compile()
    return nc


def get_program(T):
    if T not in _programs:
        _programs[T] = build_program(T)
    return _programs[T]


def make_in_maps(x, depth_k, depth_v, cos, sin, Wq, Wk, Wv, Wo, T):
    xT16 = np.ascontiguousarray(x[0].T).astype(NPBF16)
    cosT16 = np.ascontiguousarray(cos[0, 0].T).astype(NPBF16)
    sinT16 = np.ascontiguousarray(sin[0, 0].T).astype(NPBF16)
    mask16 = np.triu(np.ones((128, 128), np.float32)).astype(NPBF16)
    in_maps = []
    for c in range(N_CORES):
        wq_c = np.ascontiguousarray(
            Wq[:, 2 * c * HD: (2 * c + 2) * HD]
            .reshape(DK, 128, NQH * HD).transpose(1, 0, 2)
        ).astype(NPBF16)
        wk_c = np.ascontiguousarray(
            Wk[:, c * HD: (c + 1) * HD].reshape(DK, 128, HD).transpose(1, 0, 2)
        ).astype(NPBF16)
        wv_c = np.ascontiguousarray(
            Wv[:, c * HD: (c + 1) * HD].reshape(DK, 128, HD).transpose(1, 0, 2)
        ).astype(NPBF16)
        wo_c = np.ascontiguousarray(
            Wo[2 * c * HD: (2 * c + 2) * HD, :].reshape(NQH, HD, DM)
            .transpose(1, 0, 2)
        ).astype(NPBF16)
        kdT_c = np.ascontiguousarray(depth_k[:, 0, c].transpose(0, 2, 1)).astype(NPBF16)
        vdT_c = np.ascontiguousarray(depth_v[:, 0, c].transpose(0, 2, 1)).astype(NPBF16)
        in_maps.append(
            {
                "xT": xT16, "wq": wq_c, "wk": wk_c, "wv": wv_c, "wo": wo_c,
                "cosT": cosT16, "sinT": sinT16, "kdT": kdT_c, "vdT": vdT_c,
                "mask": mask16,
            }
        )
    return in_maps


def kernel(x, depth_k, depth_v, cos, sin, Wq, Wk, Wv, Wo):
    x = np.asarray(x, np.float32)
    T = x.shape[1]
    nc = get_program(T)
    in_maps = make_in_maps(
        x, np.asarray(depth_k, np.float32), np.asarray(depth_v, np.float32),
        np.asarray(cos, np.float32), np.asarray(sin, np.float32),
        np.asarray(Wq, np.float32), np.asarray(Wk, np.float32),
        np.asarray(Wv, np.float32), np.asarray(Wo, np.float32), T,
    )
    trace = bool(os.environ.get("MODA_TRACE"))
    res = run_bass_kernel_spmd(nc, in_maps, list(range(N_CORES)), trace=trace)
    global last_result
    last_result = res
    total = np.zeros((T, DM), np.float32)
    for c in range(N_CORES):
        total += res.results[c]["out"].astype(np.float32)
    return total.reshape(1, T, DM)


# revision 19
# speedup vs baseline: 1.1631x; 1.1631x over previous
"""MoDA attention Trainium2 kernel, 8-way head-parallel (v2).

Sharding: core c owns Q heads {2c, 2c+1} and K/V head c (their GQA group),
plus that K head's depth caches. Each core computes its heads' projections,
RoPE, joint seq+depth softmax attention, and a partial output projection
(rows 2c*128:(2c+2)*128 of Wo). Host sums the 8 partial outputs.

v2 design notes (vs v1 baseline):
- Phase A: K / Q0 / Q1 kt-outer passes with 4 psum banks each, V packed
  4-blocks-per-bank, first matmul starts ~2us in (wk + xT row0 only).
- Depth path precomputed in phase A: eu (DVE), dl (PE ones-matmul),
  wd=exp(dl) into a [1, L*TC] row, row->128-partition broadcast via DMA
  (not GpSimd), t_acc = sum_l vdT_l*wd_l on GpSimd, wdsum rows for Z.
- Phase B software pipeline per unit k: S(k) -> outproj filler ->
  Z(k-1)+O(k-1) -> epilogue(k-2), sized so Scalar exps never stall PE.
  PSUM: psS 3 + psO 2 + psZ 1 + psOut 2 = 8 banks.
- Epilogue is 2 DVE ops (o_ps + t_acc, * zinv-broadcast).
- Out tiles batched [128, DM] -> 16 output DMAs.
"""

import os
import sys

sys.path.insert(0, "/opt/trn_rl_repo")

import numpy as np
import ml_dtypes

import concourse.bass as bass
import concourse.tile as tile
import concourse.mybir as mybir
from concourse import bacc
from concourse.bass_utils import run_bass_kernel_spmd

BF16 = mybir.dt.bfloat16
FP32 = mybir.dt.float32
NPBF16 = ml_dtypes.bfloat16

HQ, HK, HD, DM = 16, 8, 128, 2048
L = 4
GQA = HQ // HK
SCALE = float(HD) ** -0.5
N_CORES = 8
NQH = 2  # Q heads per core
TC = 512  # T chunk (free dim of most matmuls)
DK = DM // 128  # contraction tiles

_programs = {}
last_result = None


def _ts(i, n):
    return bass.ts(i, n)


def build_program(T):
    nc = bacc.Bacc(
        "TRN2",
        target_bir_lowering=False,
        debug=False,
        enable_asserts=False,
        num_devices=N_CORES,
    )

    xT = nc.dram_tensor("xT", [DM, T], BF16, kind="ExternalInput").ap()
    wq = nc.dram_tensor("wq", [128, DK, NQH * HD], BF16, kind="ExternalInput").ap()
    wk = nc.dram_tensor("wk", [128, DK, HD], BF16, kind="ExternalInput").ap()
    wv = nc.dram_tensor("wv", [128, DK, HD], BF16, kind="ExternalInput").ap()
    wo = nc.dram_tensor("wo", [128, NQH, DM], BF16, kind="ExternalInput").ap()
    cosT = nc.dram_tensor("cosT", [HD, T], BF16, kind="ExternalInput").ap()
    sinT = nc.dram_tensor("sinT", [HD, T], BF16, kind="ExternalInput").ap()
    kdT = nc.dram_tensor("kdT", [L, HD, T], BF16, kind="ExternalInput").ap()
    vdT = nc.dram_tensor("vdT", [L, HD, T], BF16, kind="ExternalInput").ap()
    mask = nc.dram_tensor("mask", [128, 128], BF16, kind="ExternalInput").ap()
    out = nc.dram_tensor("out", [T, DM], BF16, kind="ExternalOutput").ap()
    NU_ = (T // TC) * NQH
    wdd = nc.dram_tensor("wdd", [NU_, L, TC], BF16, kind="Internal").ap()

    NCH = T // TC  # 4 chunks
    NTB = T // 128  # 16 blocks
    NU = NCH * NQH  # 8 units

    with tile.TileContext(nc) as tc:
        with tc.tile_pool(name="persist", bufs=1) as cp:
            # ---- persistent SBUF ----
            wq_sb = cp.tile([128, DK, NQH * HD], BF16)
            wk_sb = cp.tile([128, DK, HD], BF16)
            wv_sb = cp.tile([128, DK, HD], BF16)
            wo_sb = cp.tile([128, NQH, DM], BF16)
            mask_sb = cp.tile([128, 128], BF16)
            ones_sb = cp.tile([128, 128], BF16)
            nc.vector.memset(ones_sb[:], 1.0)
            qT_sb = cp.tile([128, NQH, T], BF16)
            kT_sb = cp.tile([128, T], BF16)
            v_sb = cp.tile([128, NTB, HD], BF16)
            oT_sb = cp.tile([128, NQH, T], BF16)
            tacc_sb = cp.tile([128, NU, TC], BF16)  # depth contrib per unit
            wd4p_sb = [cp.tile([4, TC], BF16, name=f"wd4p{u}") for u in range(NU)]

            with tc.tile_pool(name="phA_sb", bufs=1) as ap_, \
                 tc.tile_pool(name="psA", bufs=7, space="PSUM") as psA, \
                 tc.tile_pool(name="sRope", bufs=2) as sR, \
                 tc.tile_pool(name="sEu", bufs=3) as sEu, \
                 tc.tile_pool(name="sBc", bufs=2) as sBc, \
                 tc.tile_pool(name="sTt", bufs=2) as sTt:
                xT_sb = ap_.tile([128, DK, T], BF16)
                cos_sb = ap_.tile([128, T], BF16)
                sin_sb = ap_.tile([128, T], BF16)
                kdT_sb = ap_.tile([128, L, T], BF16)
                vdT_sb = ap_.tile([128, L, T], BF16)

                # ---- input DMAs: sync queue feeds the proj pipeline in
                # consumption order; scalar queue brings rope/depth data ----
                nc.sync.dma_start(wk_sb[:, 0, :], wk[:, 0, :])
                nc.sync.dma_start(xT_sb[:, 0, 0:TC], xT[_ts(0, 128), 0:TC])
                nc.sync.dma_start(xT_sb[:, 0, TC:T], xT[_ts(0, 128), TC:T])
                nc.sync.dma_start(wk_sb[:, 1:DK, :], wk[:, 1:DK, :])
                nc.sync.dma_start(wq_sb[:], wq[:])
                for kt in range(1, DK):
                    eng = nc.sync if kt % 2 == 0 else nc.scalar
                    eng.dma_start(xT_sb[:, kt, :], xT[_ts(kt, 128), :])
                nc.sync.dma_start(wv_sb[:], wv[:])
                nc.sync.dma_start(wo_sb[:], wo[:])
                nc.sync.dma_start(mask_sb[:], mask[:])
                nc.scalar.dma_start(cos_sb[:], cosT[:])
                nc.scalar.dma_start(sin_sb[:], sinT[:])
                for l in range(L):
                    nc.scalar.dma_start(kdT_sb[:, l, :], kdT[l])
                for l in range(L):
                    nc.scalar.dma_start(vdT_sb[:, l, :], vdT[l])

                def rope_chunk(ps, dst, c):
                    # dst = ps*cos + rotate_half(ps)*sin, all [128, TC]
                    cs = cos_sb[:, _ts(c, TC)]
                    sn = sin_sb[:, _ts(c, TC)]
                    praw = sR.tile([128, TC], BF16, tag="praw")
                    nc.scalar.copy(praw[:], ps[:])  # frees the psum bank fast
                    prot = sR.tile([128, TC], BF16, tag="prot")
                    nc.vector.tensor_copy(prot[0:64, :], praw[64:128, :])
                    nc.vector.tensor_copy(prot[64:128, :], praw[0:64, :])
                    t1 = sR.tile([128, TC], BF16, tag="t1")
                    nc.vector.tensor_mul(t1[:], praw[:], cs)
                    t2 = sR.tile([128, TC], BF16, tag="t2")
                    nc.vector.tensor_mul(t2[:], prot[:], sn)
                    nc.vector.tensor_tensor(
                        dst[0:64, :], t1[0:64, :], t2[0:64, :],
                        op=mybir.AluOpType.subtract,
                    )
                    nc.vector.tensor_add(dst[64:128, :], t1[64:128, :], t2[64:128, :])

                # ---- pass 1: K proj (kt-outer) + V proj interleaved per
                # xT row; V uses a [128, 4, 128] layout in each bank so each
                # bank holds one whole-row group (no interleaved start/stop
                # column groups: V matmuls write [128, 512] spans via a
                # blocked rhs). ----
                kps = [psA.tile([128, TC], FP32, tag="a", name=f"kps{c}") for c in range(NCH)]
                for kt in range(DK):
                    for c in range(NCH):
                        nc.tensor.matmul(
                            kps[c][:], wk_sb[:, kt, :], xT_sb[:, kt, _ts(c, TC)],
                            start=(kt == 0), stop=(kt == DK - 1),
                        )
                for c in range(NCH):
                    rope_chunk(kps[c], kT_sb[:, _ts(c, TC)], c)

                # ---- pass 2: V proj, 4 t-blocks packed per psum bank ----
                for g in range(NCH):
                    vp = psA.tile([128, TC], FP32, tag="a", name=f"vp{g}")
                    for tbl in range(4):
                        tb = g * 4 + tbl
                        for kt in range(DK):
                            nc.tensor.matmul(
                                vp[:, _ts(tbl, 128)],
                                xT_sb[:, kt, _ts(tb, 128)], wv_sb[:, kt, :],
                                start=(kt == 0), stop=(kt == DK - 1),
                            )
                    if g % 2 == 0:
                        nc.scalar.copy(v_sb[:, g * 4:(g + 1) * 4, :], vp[:])
                    else:
                        nc.vector.tensor_copy(v_sb[:, g * 4:(g + 1) * 4, :], vp[:])

                # depth precompute for unit (c, h): eu (DVE) -> dl (PE)
                # -> wd=exp (Scalar) -> DRAM roundtrip bcast -> tacc (DVE)
                def depth_unit(c, h):
                    # dl matmul vs an all-ones [128,128] stationary operand:
                    # every psum row gets Z_l, so exp yields the broadcast
                    # wd_l tile directly (no DMA broadcast needed).
                    u = c * NQH + h
                    bcs = sBc.tile([128, L, TC], BF16, tag="bcs")
                    for l in range(L):
                        eu = sEu.tile([128, TC], BF16, tag="eu")
                        nc.vector.tensor_mul(
                            eu[:], qT_sb[:, h, _ts(c, TC)],
                            kdT_sb[:, l, _ts(c, TC)],
                        )
                        dlp = psA.tile([128, TC], FP32, tag="a", name=f"dl{u}_{l}")
                        nc.tensor.matmul(
                            dlp[:], ones_sb[:], eu[:], start=True, stop=True
                        )
                        nc.scalar.activation(
                            bcs[:, l, :], dlp[:],
                            mybir.ActivationFunctionType.Exp, scale=SCALE,
                        )
                    nc.sync.dma_start(wdd[u], bcs[0:1, :, :])
                    nc.sync.dma_start(wd4p_sb[u][:], wdd[u])
                    # tacc = sum_l vdT_l * wd_l  (batched DVE)
                    tmp4 = sTt.tile([128, L, TC], BF16, tag="tmp4")
                    nc.vector.tensor_mul(
                        tmp4[:], vdT_sb[:, :, _ts(c, TC)], bcs[:]
                    )
                    ta2 = sTt.tile([128, TC], BF16, tag="ta2")
                    nc.vector.tensor_add(ta2[:], tmp4[:, 0, :], tmp4[:, 1, :])
                    ta3 = sTt.tile([128, TC], BF16, tag="ta3")
                    nc.vector.tensor_add(ta3[:], tmp4[:, 2, :], tmp4[:, 3, :])
                    nc.vector.tensor_add(tacc_sb[:, u, :], ta2[:], ta3[:])

                # ---- pass 3: Q0 proj ----
                q0ps = [psA.tile([128, TC], FP32, tag="a", name=f"q0ps{c}") for c in range(NCH)]
                for kt in range(DK):
                    for c in range(NCH):
                        nc.tensor.matmul(
                            q0ps[c][:], wq_sb[:, kt, 0:HD], xT_sb[:, kt, _ts(c, TC)],
                            start=(kt == 0), stop=(kt == DK - 1),
                        )
                for c in range(NCH):
                    rope_chunk(q0ps[c], qT_sb[:, 0, _ts(c, TC)], c)

                # ---- pass 4: Q1 proj; per-chunk rope + depth follow ----
                q1ps = [psA.tile([128, TC], FP32, tag="a", name=f"q1ps{c}") for c in range(NCH)]
                for kt in range(DK):
                    for c in range(NCH):
                        nc.tensor.matmul(
                            q1ps[c][:], wq_sb[:, kt, HD:2 * HD], xT_sb[:, kt, _ts(c, TC)],
                            start=(kt == 0), stop=(kt == DK - 1),
                        )
                for c in range(NCH):
                    rope_chunk(q1ps[c], qT_sb[:, 1, _ts(c, TC)], c)
                for c in range(NCH):
                    depth_unit(c, 0)
                    depth_unit(c, 1)

            # ---- phase B: attention with software pipeline ----
            with tc.tile_pool(name="psS", bufs=3, space="PSUM") as psS, \
                 tc.tile_pool(name="psO", bufs=2, space="PSUM") as psO, \
                 tc.tile_pool(name="psZ", bufs=1, space="PSUM") as psZ, \
                 tc.tile_pool(name="psOut", bufs=2, space="PSUM") as psOut, \
                 tc.tile_pool(name="sU", bufs=34) as sU, \
                 tc.tile_pool(name="sZb", bufs=2) as sZb, \
                 tc.tile_pool(name="sOs", bufs=2) as sOs, \
                 tc.tile_pool(name="sRes", bufs=2) as sRes:

                units = [(c, h) for c in range(NCH) for h in range(NQH)]

                def s_phase(k):
                    c, h = units[k]
                    jmax = (c + 1) * 4
                    c0 = c * 4
                    us = []
                    for jb in range(jmax):
                        off = max(0, jb - c0) * 128
                        sp = psS.tile([128, TC], FP32, tag="s")
                        nc.tensor.matmul(
                            sp[:, off:TC], kT_sb[:, _ts(jb, 128)],
                            qT_sb[:, h, c * TC + off:(c + 1) * TC],
                            start=True, stop=True,
                        )
                        uu = sU.tile([128, TC], BF16, tag="u")
                        nc.scalar.activation(
                            uu[:, off:TC], sp[:, off:TC],
                            mybir.ActivationFunctionType.Exp, scale=SCALE,
                        )
                        if jb >= c0:
                            nc.vector.tensor_mul(
                                uu[:, off:off + 128], uu[:, off:off + 128],
                                mask_sb[:],
                            )
                        us.append((jb, off, uu))
                    return us

                def z_phase(k, us):
                    zp = psZ.tile([128, TC], FP32, tag="z")
                    for jb, off, uu in us:
                        nc.tensor.matmul(
                            zp[:, off:TC], ones_sb[:], uu[:, off:TC],
                            start=(jb == 0), stop=False,
                        )
                    nc.tensor.matmul(
                        zp[:], ones_sb[0:4, :], wd4p_sb[k][:],
                        start=False, stop=True,
                    )
                    zb = sZb.tile([128, TC], FP32, tag="zb")
                    nc.vector.reciprocal_approx_fast(zb[:], zp[:])
                    return zb

                def o_phase(k, us):
                    op = psO.tile([128, TC], FP32, tag="o")
                    for jb, off, uu in us:
                        nc.tensor.matmul(
                            op[:, off:TC], v_sb[:, jb, :], uu[:, off:TC],
                            start=(jb == 0), stop=(jb == len(us) - 1),
                        )
                    return op

                def epilogue(k, op, zb):
                    c, h = units[k]
                    osum = sOs.tile([128, TC], FP32, tag="osum")
                    nc.vector.tensor_add(osum[:], op[:], tacc_sb[:, k, :])
                    nc.vector.tensor_mul(
                        oT_sb[:, h, _ts(c, TC)], osum[:], zb[:]
                    )

                ncopy = [0]

                def outproj_tb(tb, split_dma=False):
                    res = sRes.tile([128, DM], BF16, tag="res")
                    for nch in range(DM // TC):
                        opp = psOut.tile([128, TC], FP32, tag="op")
                        for h in range(NQH):
                            nc.tensor.matmul(
                                opp[:], oT_sb[:, h, _ts(tb, 128)],
                                wo_sb[:, h, _ts(nch, TC)],
                                start=(h == 0), stop=(h == NQH - 1),
                            )
                        ncopy[0] += 1
                        if ncopy[0] % 2 == 0:
                            nc.scalar.copy(res[:, _ts(nch, TC)], opp[:])
                        else:
                            nc.vector.tensor_copy(res[:, _ts(nch, TC)], opp[:])
                        if split_dma:
                            nc.sync.dma_start(
                                out[_ts(tb, 128), _ts(nch, TC)], res[:, _ts(nch, TC)]
                            )
                    if not split_dma:
                        nc.sync.dma_start(out[_ts(tb, 128), :], res[:])

                pend_out = []

                def drain_out(n):
                    while n > 0 and pend_out:
                        outproj_tb(pend_out.pop(0))
                        n -= 1

                saved = {}
                for k in range(NU):
                    saved[k] = (s_phase(k),)
                    drain_out(1)
                    if k > 0:
                        us_prev = saved[k - 1][0]
                        zb = z_phase(k - 1, us_prev)
                        drain_out(1)
                        op = o_phase(k - 1, us_prev)
                        saved[k - 1] = (op, zb)
                    if k > 1:
                        op, zb = saved.pop(k - 2)
                        epilogue(k - 2, op, zb)
                        c2, h2 = units[k - 2]
                        if h2 == NQH - 1:
                            pend_out.extend(range(c2 * 4, (c2 + 1) * 4))
                # drain: unit 7 z/o, epilogues 6, 7, remaining outproj
                us7 = saved[7][0]
                zb7 = z_phase(7, us7)
                drain_out(2)
                op7 = o_phase(7, us7)
                op6, zb6 = saved.pop(6)
                epilogue(6, op6, zb6)
                drain_out(2)
                epilogue(7, op7, zb7)
                for tb in range(12, 16):
                    outproj_tb(tb, split_dma=(tb == 15))

    nc.compile()
    return nc


def get_program(T):
    if T not in _programs:
        _programs[T] = build_program(T)
    return _programs[T]


def make_in_maps(x, depth_k, depth_v, cos, sin, Wq, Wk, Wv, Wo, T):
    xT16 = np.ascontiguousarray(x[0].T).astype(NPBF16)
    cosT16 = np.ascontiguousarray(cos[0, 0].T).astype(NPBF16)
    sinT16 = np.ascontiguousarray(sin[0, 0].T).astype(NPBF16)
    mask16 = np.triu(np.ones((128, 128), np.float32)).astype(NPBF16)
    in_maps = []
    for c in range(N_CORES):
        wq_c = np.ascontiguousarray(
            Wq[:, 2 * c * HD: (2 * c + 2) * HD]
            .reshape(DK, 128, NQH * HD).transpose(1, 0, 2)
        ).astype(NPBF16)
        wk_c = np.ascontiguousarray(
            Wk[:, c * HD: (c + 1) * HD].reshape(DK, 128, HD).transpose(1, 0, 2)
        ).astype(NPBF16)
        wv_c = np.ascontiguousarray(
            Wv[:, c * HD: (c + 1) * HD].reshape(DK, 128, HD).transpose(1, 0, 2)
        ).astype(NPBF16)
        wo_c = np.ascontiguousarray(
            Wo[2 * c * HD: (2 * c + 2) * HD, :].reshape(NQH, HD, DM)
            .transpose(1, 0, 2)
        ).astype(NPBF16)
        kdT_c = np.ascontiguousarray(depth_k[:, 0, c].transpose(0, 2, 1)).astype(NPBF16)
        vdT_c = np.ascontiguousarray(depth_v[:, 0, c].transpose(0, 2, 1)).astype(NPBF16)
        in_maps.append(
            {
                "xT": xT16, "wq": wq_c, "wk": wk_c, "wv": wv_c, "wo": wo_c,
                "cosT": cosT16, "sinT": sinT16, "kdT": kdT_c, "vdT": vdT_c,
                "mask": mask16,
            }
        )
    return in_maps


def kernel(x, depth_k, depth_v, cos, sin, Wq, Wk, Wv, Wo):
    x = np.asarray(x, np.float32)
    T = x.shape[1]
    nc = get_program(T)
    in_maps = make_in_maps(
        x, np.asarray(depth_k, np.float32), np.asarray(depth_v, np.float32),
        np.asarray(cos, np.float32), np.asarray(sin, np.float32),
        np.asarray(Wq, np.float32), np.asarray(Wk, np.float32),
        np.asarray(Wv, np.float32), np.asarray(Wo, np.float32), T,
    )
    trace = bool(os.environ.get("MODA_TRACE"))
    res = run_bass_kernel_spmd(nc, in_maps, list(range(N_CORES)), trace=trace)
    global last_result
    last_result = res
    total = np.zeros((T, DM), np.float32)
    for c in range(N_CORES):
        total += res.results[c]["out"].astype(np.float32)
    return total.reshape(1, T, DM)
